# revision 36
# baseline (speedup 1.0000x reference)
"""Trainium2 Bass kernel for nn_ContrastiveLoss (survival contrastive loss).

Symmetric-pair strategy (8 NeuronCores, SPMD single program):
  - Host sorts rows by survival time; exp(sim) is symmetric, so each
    unordered 128-row-tile pair {A, B} is computed exactly once.  Core c
    owns tiles A with A = 8k + c (k = 0..7) and for each computes the
    diagonal tile (upper triangle only) plus d = 1..32 (k < 4) or
    d = 1..31 (k >= 4) tiles to the right (mod 64).
  - Row-direction sums come from scalar-accum / vector reduces; the
    column-direction sums (the transposed half of each pair) come from
    M=1 ones-matmuls on the tensor engine, staged in PSUM waves and
    DMA'd to DRAM raw.  The host adds row and column partials.
  - Positives (|t_i - t_j| < 365) live within 1216 sorted rows of the
    diagonal, so the numerator (mask * e, both directions) only touches
    the first 1408 e-columns of each tile-row.  Host fixes up any row
    whose window escapes the band (never for uniform survival times).
"""
import numpy as np
from contextlib import ExitStack

import ml_dtypes

import concourse.bass as bass
import concourse.tile as tile
from concourse import bacc, mybir
from concourse import bass_utils

F32 = mybir.dt.float32
BF16 = mybir.dt.bfloat16

B = 8192
D = 128
NCORES = 8
K = 8                      # A-tiles per core
MASKW = 1408               # numerator band width (11 tiles)
MARGIN = 1216              # guaranteed one-side window coverage
MOFF = 4224                # colp offset of the masked colsum region
CREG = 6144                # colp region size per k (6 waves of 1024)
NEG = -1e9
SHIFT = 10.0
SQRT_INV_T = float(np.sqrt(10.0))

_CACHE: dict = {}
_RUN_KW: dict = {}
_LAST_EXEC_NS = None
_LAST_RES = None


def _ew(k):
    """e-columns for tile-row k: 128 diag + 128*(32 or 31) wide."""
    return 4224 if k < 4 else 4096


def _mm_chunks(b0, b1, base):
    """split [b0,b1) e-cols into matmul chunks: 512-grid + zt wrap."""
    out = []
    off = b0
    while off < b1:
        n = min(512 - (off % 512), b1 - off)
        zl = (base + off) % B
        n = min(n, B - zl)
        out.append((off, zl, n))
        off += n
    return out


def _colp_chunks(k):
    """512-wide colsum chunks: (chunk_idx, [(flat_off, src, src_off, n)]).
    flat layout per k: [0, ew) unmasked colsums, [ew, ew+1408) masked."""
    ew = _ew(k)
    cw = ew + MASKW
    out = []
    for i in range((cw + 511) // 512):
        f0, f1 = 512 * i, min(512 * i + 512, cw)
        segs = []
        if f0 < ew:
            segs.append((f0, "e", f0, min(f1, ew) - f0))
        if f1 > ew:
            s = max(f0, ew)
            segs.append((s, "junk", s - ew, f1 - s))
        out.append((i, segs))
    return out


def _strip_ap(t):
    """AP over partitions {0, 32, 64, 96} x full free dim of tile t."""
    a = t[:]
    return bass.AP(tensor=a.tensor, offset=a.offset,
                   ap=[[a.ap[0][0] * 32, 4]] + [list(p) for p in a.ap[1:]])


def _build_program():
    nc = bacc.Bacc("TRN2", target_bir_lowering=False, debug=False,
                   num_devices=NCORES)

    d_zt = nc.dram_tensor("zt", [128, B], BF16, kind="ExternalInput").ap()
    d_tri = nc.dram_tensor("tri", [128, 128], F32, kind="ExternalInput").ap()
    d_mask = nc.dram_tensor("mask", [128, K, MASKW], BF16,
                            kind="ExternalInput").ap()
    # res[p, k] = row-direction s_all partial, res[p, 8+k] = s_pos partial
    d_res = nc.dram_tensor("res", [128, 2 * K], F32,
                           kind="ExternalOutput").ap()
    # colp[k, 32*j, 512*b:+512]: chunk i = 4b + j of the per-k flat colsum
    # vector ([0, ew): unmasked, [ew, ew+1408): masked); others garbage
    d_colp = nc.dram_tensor("colp", [K, 128, 1536], BF16,
                            kind="ExternalOutput").ap()

    with tile.TileContext(nc) as tc, ExitStack() as ctx:
        consts = ctx.enter_context(tc.tile_pool(name="consts", bufs=1))
        big = ctx.enter_context(tc.tile_pool(name="big", bufs=1))
        epool = ctx.enter_context(tc.tile_pool(name="epool", bufs=2))
        mpool = ctx.enter_context(tc.tile_pool(name="mpool", bufs=2))
        jpool = ctx.enter_context(tc.tile_pool(name="jpool", bufs=2))
        j2pool = ctx.enter_context(tc.tile_pool(name="j2pool", bufs=2))
        stats = ctx.enter_context(tc.tile_pool(name="stats", bufs=2))
        psp = ctx.enter_context(tc.tile_pool(name="psp", bufs=2, space="PSUM"))
        psc = ctx.enter_context(tc.tile_pool(name="psc", bufs=1, space="PSUM"))
        cstg = ctx.enter_context(tc.tile_pool(name="cstg", bufs=2))

        tri_neg = consts.tile([128, 128], F32)
        nc.sync.dma_start(out=tri_neg[:], in_=d_tri[:])
        bias_shift = consts.tile([128, 1], F32)
        nc.gpsimd.memset(bias_shift[:], -SHIFT)
        ones = consts.tile([128, 1], BF16)
        nc.gpsimd.memset(ones[:], 1.0)

        zt = big.tile([128, B], BF16)
        res = big.tile([128, 2 * K], F32)
        for h in range(8):
            nc.sync.dma_start(out=zt[:, h * 1024:(h + 1) * 1024],
                              in_=d_zt[:, h * 1024:(h + 1) * 1024])

        for k in range(K):
            ewk = _ew(k)
            base = 8 * k * 128
            lhsT = zt[:, base:base + 128]
            nblk = (ewk + 1023) // 1024
            e = epool.tile([128, 4224], BF16, tag="e")
            m = mpool.tile([128, MASKW], BF16, tag="m")
            nc.sync.dma_start(out=m[:], in_=d_mask[:, k, :])
            sacc = stats.tile([128, 8], F32, tag="sacc")

            for bi in range(nblk):
                b0 = bi * 1024
                b1 = min(b0 + 1024, ewk)
                bw = b1 - b0
                ps = psp.tile([128, 1024], F32, tag="ps")
                for off, zl, n in _mm_chunks(b0, b1, base):
                    nc.tensor.matmul(ps[:, off - b0:off - b0 + n],
                                     lhsT=lhsT, rhs=zt[:, zl:zl + n],
                                     start=True, stop=True)
                if bi == 0:
                    nc.vector.tensor_add(ps[:, 0:128], ps[:, 0:128],
                                         tri_neg[:])
                if bi < 3:
                    nc.scalar.activation(out=e[:, b0:b1], in_=ps[:, :bw],
                                         func=mybir.ActivationFunctionType.Exp,
                                         bias=bias_shift[:], scale=1.0,
                                         accum_out=sacc[:, bi:bi + 1])
                else:
                    nc.scalar.activation(out=e[:, b0:b1], in_=ps[:, :bw],
                                         func=mybir.ActivationFunctionType.Exp,
                                         bias=bias_shift[:], scale=1.0)
                    junk2 = j2pool.tile([128, 1152], BF16, tag="junk2")
                    nc.vector.tensor_scalar(
                        out=junk2[:, :bw], in0=e[:, b0:b1],
                        scalar1=1.0, scalar2=None, op0=mybir.AluOpType.mult,
                        op1=mybir.AluOpType.add,
                        accum_out=sacc[:, bi:bi + 1])

            # numerator row-direction: fused mask*e + row-sum on the band
            junk = jpool.tile([128, MASKW], BF16, tag="junk")
            nc.vector.scalar_tensor_tensor(
                out=junk[:], in0=m[:], scalar=1.0, in1=e[:, 0:MASKW],
                op0=mybir.AluOpType.mult, op1=mybir.AluOpType.mult,
                accum_out=res[:, K + k:K + k + 1])
            nc.vector.tensor_reduce(out=res[:, k:k + 1],
                                    in_=sacc[:, :nblk],
                                    axis=mybir.AxisListType.X,
                                    op=mybir.AluOpType.add)

            # column-direction sums: M=1 ones-matmuls packed 4 chunks per
            # PSUM bank via tile_position col-strips; one copy + DMA per k
            cb = psc.tile([128, 1536], F32, tag="cb")
            for i, segs in _colp_chunks(k):
                j = (i % 4) * 32
                sb = 512 * (i // 4)
                for f, src, so, n in segs:
                    rhs = (e[:, so:so + n] if src == "e"
                           else junk[:, so:so + n])
                    o = sb + (f - 512 * i)
                    nc.tensor.matmul(cb[j:j + 1, o:o + n],
                                     lhsT=ones[:], rhs=rhs,
                                     tile_position=(0, j),
                                     start=True, stop=True)
            cs = cstg.tile([128, 1536], BF16, tag="cs")
            nc.vector.tensor_copy(cs[:], cb[:])
            nc.sync.dma_start(out=d_colp[k, :, :], in_=cs[:])

        nc.sync.dma_start(out=d_res[:], in_=res[:])

    nc.compile()
    return nc


def _get_program():
    if "nc" not in _CACHE:
        _CACHE["nc"] = _build_program()
    return _CACHE["nc"]


def _host_prep(emb, t_i):
    perm = np.argsort(t_i, kind="stable")
    t_s = t_i[perm]
    emb_s = emb[perm]
    nrm = np.maximum(np.sqrt((emb_s.astype(np.float64) ** 2).sum(axis=1)),
                     1e-12)
    z = emb_s / nrm[:, None]
    zT = np.ascontiguousarray((z * SQRT_INV_T).T.astype(ml_dtypes.bfloat16))
    tri = np.where(np.arange(128)[:, None] >= np.arange(128)[None, :],
                   np.float32(NEG), np.float32(0.0))
    return perm, t_s, z, zT, tri


def _in_maps(zT, t_s, tri):
    t_sf = t_s.astype(np.float32)
    col_idx = np.arange(MASKW)
    maps = []
    for c in range(NCORES):
        zt_c = np.ascontiguousarray(np.roll(zT, -c * 128, axis=1))
        mask_c = np.empty((128, K, MASKW), dtype=ml_dtypes.bfloat16)
        for k in range(K):
            g0 = (8 * k + c) * 128
            rows = t_sf[g0:g0 + 128]
            cols = t_sf[(g0 + col_idx) % B]
            mask_c[:, k, :] = (
                np.abs(rows[:, None] - cols[None, :]) < 365.0
            ).astype(ml_dtypes.bfloat16)
        maps.append({"zt": zt_c, "mask": mask_c,
                     "tri": np.ascontiguousarray(tri)})
    return maps


def _combine(results, t_s, cen_s, z):
    s_all = np.zeros(B, np.float64)
    s_pos = np.zeros(B, np.float64)
    for c in range(NCORES):
        r = np.asarray(results[c]["res"], dtype=np.float64)
        craw = np.asarray(results[c]["colp"], dtype=np.float64)
        # [K, 128, 1536] -> flat [K, 5632]: chunk i=4b+j at [k, 32j, 512b:]
        strips = craw[:, ::32, :].reshape(K, 4, 3, 512)
        colp = strips.transpose(0, 2, 1, 3).reshape(K, -1)
        for k in range(K):
            g0 = (8 * k + c) * 128
            rows = g0 + np.arange(128)
            s_all[rows] += r[:, k]
            s_pos[rows] += r[:, K + k]
            ewk = _ew(k)
            cols = (g0 + np.arange(ewk)) % B
            s_all[cols] += colp[k, 0:ewk]
            mcols = (g0 + np.arange(MASKW)) % B
            s_pos[mcols] += colp[k, ewk:ewk + MASKW]

    lo = np.searchsorted(t_s, t_s - 364, side="left")
    hi = np.searchsorted(t_s, t_s + 364, side="right")
    has_pos = ((hi - lo - 1) > 0) & (cen_s == 1)

    g = np.arange(B)
    bad = has_pos & ((lo < g - MARGIN) | (hi > g + 1 + MARGIN))
    if np.any(bad):
        zs = (z * SQRT_INV_T).astype(np.float32)
        for i in np.nonzero(bad)[0]:
            extra = list(range(lo[i], i - MARGIN)) + \
                    list(range(i + 1 + MARGIN, hi[i]))
            js = np.array([j for j in extra if j != i], dtype=np.int64)
            if js.size:
                sims = zs[i] @ zs[js].T
                s_pos[i] += np.exp(sims - SHIFT).sum()

    cnt = float(has_pos.sum())
    if cnt <= 0:
        return np.float32(0.0)
    ratio = np.where(has_pos, s_all / np.maximum(s_pos, 1e-300), 1.0)
    per_row = np.where(has_pos, np.log(ratio), 0.0)
    return np.float32(per_row.sum() / max(cnt, 1.0))


def kernel(embeddings, survival_times, censor):
    emb = np.asarray(embeddings, dtype=np.float32)
    t_i = np.asarray(survival_times).astype(np.int64)
    cen = np.asarray(censor).astype(np.int64)
    assert emb.shape == (B, D)

    perm, t_s, z, zT, tri = _host_prep(emb, t_i)
    cen_s = cen[perm]
    nc = _get_program()
    maps = _in_maps(zT, t_s, tri)
    res = bass_utils.run_bass_kernel_spmd(nc, maps,
                                          core_ids=list(range(NCORES)),
                                          **_RUN_KW)
    global _LAST_EXEC_NS, _LAST_RES
    _LAST_EXEC_NS = res.exec_time_ns
    _LAST_RES = res
    return _combine(res.results, t_s, cen_s, z)


# revision 37
# speedup vs baseline: 1.1539x; 1.1539x over previous
"""Trainium2 Bass kernel for nn_ContrastiveLoss (survival contrastive loss).

Symmetric-pair strategy (8 NeuronCores, SPMD single program):
  - Host sorts rows by survival time; exp(sim) is symmetric, so each
    unordered 128-row-tile pair {A, B} is computed exactly once.  Core c
    owns tiles A with A = 8k + c (k = 0..7) and for each computes the
    diagonal tile (upper triangle only) plus d = 1..32 (k < 4) or
    d = 1..31 (k >= 4) tiles to the right (mod 64).
  - Row-direction sums come from scalar-accum / vector reduces; the
    column-direction sums (the transposed half of each pair) come from
    M=1 ones-matmuls on the tensor engine, staged in PSUM waves and
    DMA'd to DRAM raw.  The host adds row and column partials.
  - Positives (|t_i - t_j| < 365) live within 1216 sorted rows of the
    diagonal, so the numerator (mask * e, both directions) only touches
    the first 1408 e-columns of each tile-row.  Host fixes up any row
    whose window escapes the band (never for uniform survival times).
"""
import numpy as np
from contextlib import ExitStack

import ml_dtypes

import concourse.bass as bass
import concourse.tile as tile
from concourse import bacc, mybir
from concourse import bass_utils

F32 = mybir.dt.float32
BF16 = mybir.dt.bfloat16

B = 8192
D = 128
NCORES = 8
K = 8                      # A-tiles per core
MASKW = 1408               # numerator band width (11 tiles)
MARGIN = 1216              # guaranteed one-side window coverage
MOFF = 4224                # colp offset of the masked colsum region
CREG = 6144                # colp region size per k (6 waves of 1024)
NEG = -1e9
SHIFT = 10.0
SQRT_INV_T = float(np.sqrt(10.0))

_CACHE: dict = {}
_RUN_KW: dict = {}
_LAST_EXEC_NS = None
_LAST_RES = None


def _ew(k):
    """e-columns for tile-row k: 128 diag + 128*(32 or 31) wide."""
    return 4224 if k < 4 else 4096


def _mm_chunks(b0, b1, base):
    """split [b0,b1) e-cols into matmul chunks: 512-grid + zt wrap."""
    out = []
    off = b0
    while off < b1:
        n = min(512 - (off % 512), b1 - off)
        zl = (base + off) % B
        n = min(n, B - zl)
        out.append((off, zl, n))
        off += n
    return out


def _colp_chunks(k):
    """512-wide colsum chunks: (chunk_idx, [(flat_off, src, src_off, n)]).
    flat layout per k: [0, ew) unmasked colsums, [ew, ew+1408) masked."""
    ew = _ew(k)
    cw = ew + MASKW
    out = []
    for i in range((cw + 511) // 512):
        f0, f1 = 512 * i, min(512 * i + 512, cw)
        segs = []
        if f0 < ew:
            segs.append((f0, "e", f0, min(f1, ew) - f0))
        if f1 > ew:
            s = max(f0, ew)
            segs.append((s, "junk", s - ew, f1 - s))
        out.append((i, segs))
    return out


def _strip_ap(t):
    """AP over partitions {0, 32, 64, 96} x full free dim of tile t."""
    a = t[:]
    return bass.AP(tensor=a.tensor, offset=a.offset,
                   ap=[[a.ap[0][0] * 32, 4]] + [list(p) for p in a.ap[1:]])


def _build_program():
    nc = bacc.Bacc("TRN2", target_bir_lowering=False, debug=False,
                   num_devices=NCORES)

    d_zt = nc.dram_tensor("zt", [128, B], BF16, kind="ExternalInput").ap()
    d_tri = nc.dram_tensor("tri", [128, 128], F32, kind="ExternalInput").ap()
    d_mask = nc.dram_tensor("mask", [128, K, MASKW], BF16,
                            kind="ExternalInput").ap()
    # res[p, k] = row-direction s_all partial, res[p, 8+k] = s_pos partial
    d_res = nc.dram_tensor("res", [128, 2 * K], F32,
                           kind="ExternalOutput").ap()
    # colp[k, 32*j, 512*b:+512]: chunk i = 4b + j of the per-k flat colsum
    # vector ([0, ew): unmasked, [ew, ew+1408): masked); others garbage
    d_colp = nc.dram_tensor("colp", [K, 128, 1536], F32,
                            kind="ExternalOutput").ap()

    with tile.TileContext(nc) as tc, ExitStack() as ctx:
        consts = ctx.enter_context(tc.tile_pool(name="consts", bufs=1))
        big = ctx.enter_context(tc.tile_pool(name="big", bufs=1))
        epool = ctx.enter_context(tc.tile_pool(name="epool", bufs=2))
        mpool = ctx.enter_context(tc.tile_pool(name="mpool", bufs=2))
        jpool = ctx.enter_context(tc.tile_pool(name="jpool", bufs=2))
        j2pool = ctx.enter_context(tc.tile_pool(name="j2pool", bufs=2))
        stats = ctx.enter_context(tc.tile_pool(name="stats", bufs=2))
        psp = ctx.enter_context(tc.tile_pool(name="psp", bufs=2, space="PSUM"))
        psc = ctx.enter_context(tc.tile_pool(name="psc", bufs=1, space="PSUM"))
        cstg = ctx.enter_context(tc.tile_pool(name="cstg", bufs=2))

        tri_neg = consts.tile([128, 128], F32)
        nc.sync.dma_start(out=tri_neg[:], in_=d_tri[:])
        bias_shift = consts.tile([128, 1], F32)
        nc.gpsimd.memset(bias_shift[:], -SHIFT)
        ones = consts.tile([128, 1], BF16)
        nc.gpsimd.memset(ones[:], 1.0)

        zt = big.tile([128, B], BF16)
        res = big.tile([128, 2 * K], F32)
        for h in range(8):
            nc.sync.dma_start(out=zt[:, h * 1024:(h + 1) * 1024],
                              in_=d_zt[:, h * 1024:(h + 1) * 1024])

        for k in range(K):
            ewk = _ew(k)
            base = 8 * k * 128
            lhsT = zt[:, base:base + 128]
            nblk = (ewk + 1023) // 1024
            e = epool.tile([128, 4224], BF16, tag="e")
            m = mpool.tile([128, MASKW], BF16, tag="m")
            nc.sync.dma_start(out=m[:], in_=d_mask[:, k, :])
            sacc = stats.tile([128, 8], F32, tag="sacc")

            for bi in range(nblk):
                b0 = bi * 1024
                b1 = min(b0 + 1024, ewk)
                bw = b1 - b0
                ps = psp.tile([128, 1024], F32, tag="ps")
                for off, zl, n in _mm_chunks(b0, b1, base):
                    nc.tensor.matmul(ps[:, off - b0:off - b0 + n],
                                     lhsT=lhsT, rhs=zt[:, zl:zl + n],
                                     start=True, stop=True)
                if bi == 0:
                    nc.vector.tensor_add(ps[:, 0:128], ps[:, 0:128],
                                         tri_neg[:])
                if bi < 3:
                    nc.scalar.activation(out=e[:, b0:b1], in_=ps[:, :bw],
                                         func=mybir.ActivationFunctionType.Exp,
                                         bias=bias_shift[:], scale=1.0,
                                         accum_out=sacc[:, bi:bi + 1])
                else:
                    nc.scalar.activation(out=e[:, b0:b1], in_=ps[:, :bw],
                                         func=mybir.ActivationFunctionType.Exp,
                                         bias=bias_shift[:], scale=1.0)
                    junk2 = j2pool.tile([128, 1024], BF16, tag="junk2")
                    nc.vector.tensor_scalar(
                        out=junk2[:, :bw], in0=e[:, b0:b1],
                        scalar1=1.0, scalar2=None, op0=mybir.AluOpType.mult,
                        op1=mybir.AluOpType.add,
                        accum_out=sacc[:, bi:bi + 1])

            # numerator row-direction: fused mask*e + row-sum on the band
            junk = jpool.tile([128, MASKW], BF16, tag="junk")
            nc.vector.scalar_tensor_tensor(
                out=junk[:], in0=m[:], scalar=1.0, in1=e[:, 0:MASKW],
                op0=mybir.AluOpType.mult, op1=mybir.AluOpType.mult,
                accum_out=res[:, K + k:K + k + 1])
            nc.vector.tensor_reduce(out=res[:, k:k + 1],
                                    in_=sacc[:, :nblk],
                                    axis=mybir.AxisListType.X,
                                    op=mybir.AluOpType.add)

            # column-direction sums: M=1 ones-matmuls packed 4 chunks per
            # PSUM bank via tile_position col-strips; one copy + DMA per k
            cb = psc.tile([128, 1536], F32, tag="cb")
            for i, segs in _colp_chunks(k):
                j = (i % 4) * 32
                sb = 512 * (i // 4)
                for f, src, so, n in segs:
                    rhs = (e[:, so:so + n] if src == "e"
                           else junk[:, so:so + n])
                    o = sb + (f - 512 * i)
                    nc.tensor.matmul(cb[j:j + 1, o:o + n],
                                     lhsT=ones[:], rhs=rhs,
                                     tile_position=(0, j),
                                     start=True, stop=True)
            cs = cstg.tile([128, 1536], F32, tag="cs")
            nc.vector.tensor_copy(cs[:], cb[:])
            nc.sync.dma_start(out=d_colp[k, :, :], in_=cs[:])

        nc.sync.dma_start(out=d_res[:], in_=res[:])

    nc.compile()
    return nc


def _get_program():
    if "nc" not in _CACHE:
        _CACHE["nc"] = _build_program()
    return _CACHE["nc"]


def _host_prep(emb, t_i):
    perm = np.argsort(t_i, kind="stable")
    t_s = t_i[perm]
    emb_s = emb[perm]
    nrm = np.maximum(np.sqrt((emb_s.astype(np.float64) ** 2).sum(axis=1)),
                     1e-12)
    z = emb_s / nrm[:, None]
    zT = np.ascontiguousarray((z * SQRT_INV_T).T.astype(ml_dtypes.bfloat16))
    tri = np.where(np.arange(128)[:, None] >= np.arange(128)[None, :],
                   np.float32(NEG), np.float32(0.0))
    return perm, t_s, z, zT, tri


def _in_maps(zT, t_s, tri):
    t_sf = t_s.astype(np.float32)
    col_idx = np.arange(MASKW)
    maps = []
    for c in range(NCORES):
        zt_c = np.ascontiguousarray(np.roll(zT, -c * 128, axis=1))
        mask_c = np.empty((128, K, MASKW), dtype=ml_dtypes.bfloat16)
        for k in range(K):
            g0 = (8 * k + c) * 128
            rows = t_sf[g0:g0 + 128]
            cols = t_sf[(g0 + col_idx) % B]
            mask_c[:, k, :] = (
                np.abs(rows[:, None] - cols[None, :]) < 365.0
            ).astype(ml_dtypes.bfloat16)
        maps.append({"zt": zt_c, "mask": mask_c,
                     "tri": np.ascontiguousarray(tri)})
    return maps


def _combine(results, t_s, cen_s, z):
    s_all = np.zeros(B, np.float64)
    s_pos = np.zeros(B, np.float64)
    for c in range(NCORES):
        r = np.asarray(results[c]["res"], dtype=np.float64)
        craw = np.asarray(results[c]["colp"], dtype=np.float64)
        # [K, 128, 1536] -> flat [K, 5632]: chunk i=4b+j at [k, 32j, 512b:]
        strips = craw[:, ::32, :].reshape(K, 4, 3, 512)
        colp = strips.transpose(0, 2, 1, 3).reshape(K, -1)
        for k in range(K):
            g0 = (8 * k + c) * 128
            rows = g0 + np.arange(128)
            s_all[rows] += r[:, k]
            s_pos[rows] += r[:, K + k]
            ewk = _ew(k)
            cols = (g0 + np.arange(ewk)) % B
            s_all[cols] += colp[k, 0:ewk]
            mcols = (g0 + np.arange(MASKW)) % B
            s_pos[mcols] += colp[k, ewk:ewk + MASKW]

    lo = np.searchsorted(t_s, t_s - 364, side="left")
    hi = np.searchsorted(t_s, t_s + 364, side="right")
    has_pos = ((hi - lo - 1) > 0) & (cen_s == 1)

    g = np.arange(B)
    bad = has_pos & ((lo < g - MARGIN) | (hi > g + 1 + MARGIN))
    if np.any(bad):
        zs = (z * SQRT_INV_T).astype(np.float32)
        for i in np.nonzero(bad)[0]:
            extra = list(range(lo[i], i - MARGIN)) + \
                    list(range(i + 1 + MARGIN, hi[i]))
            js = np.array([j for j in extra if j != i], dtype=np.int64)
            if js.size:
                sims = zs[i] @ zs[js].T
                s_pos[i] += np.exp(sims - SHIFT).sum()

    cnt = float(has_pos.sum())
    if cnt <= 0:
        return np.float32(0.0)
    ratio = np.where(has_pos, s_all / np.maximum(s_pos, 1e-300), 1.0)
    per_row = np.where(has_pos, np.log(ratio), 0.0)
    return np.float32(per_row.sum() / max(cnt, 1.0))


def kernel(embeddings, survival_times, censor):
    emb = np.asarray(embeddings, dtype=np.float32)
    t_i = np.asarray(survival_times).astype(np.int64)
    cen = np.asarray(censor).astype(np.int64)
    assert emb.shape == (B, D)

    perm, t_s, z, zT, tri = _host_prep(emb, t_i)
    cen_s = cen[perm]
    nc = _get_program()
    maps = _in_maps(zT, t_s, tri)
    res = bass_utils.run_bass_kernel_spmd(nc, maps,
                                          core_ids=list(range(NCORES)),
                                          **_RUN_KW)
    global _LAST_EXEC_NS, _LAST_RES
    _LAST_EXEC_NS = res.exec_time_ns
    _LAST_RES = res
    return _combine(res.results, t_s, cen_s, z)


# revision 38
# speedup vs baseline: 1.1902x; 1.0314x over previous
"""Trainium2 Bass kernel for nn_ContrastiveLoss (survival contrastive loss).

Symmetric-pair strategy (8 NeuronCores, SPMD single program):
  - Host sorts rows by survival time; exp(sim) is symmetric, so each
    unordered 128-row-tile pair {A, B} is computed exactly once.  Core c
    owns tiles A with A = 8k + c (k = 0..7) and for each computes the
    diagonal tile (upper triangle only) plus d = 1..32 (k < 4) or
    d = 1..31 (k >= 4) tiles to the right (mod 64).
  - Row-direction sums come from scalar-accum / vector reduces; the
    column-direction sums (the transposed half of each pair) come from
    M=1 ones-matmuls on the tensor engine, staged in PSUM waves and
    DMA'd to DRAM raw.  The host adds row and column partials.
  - Positives (|t_i - t_j| < 365) live within 1216 sorted rows of the
    diagonal, so the numerator (mask * e, both directions) only touches
    the first 1408 e-columns of each tile-row.  Host fixes up any row
    whose window escapes the band (never for uniform survival times).
"""
import numpy as np
from contextlib import ExitStack

import ml_dtypes

import concourse.bass as bass
import concourse.tile as tile
from concourse import bacc, mybir
from concourse import bass_utils

F32 = mybir.dt.float32
BF16 = mybir.dt.bfloat16

B = 8192
D = 128
NCORES = 8
K = 8                      # A-tiles per core
MASKW = 1408               # numerator band width (11 tiles)
MARGIN = 1216              # guaranteed one-side window coverage
MOFF = 4224                # colp offset of the masked colsum region
CREG = 6144                # colp region size per k (6 waves of 1024)
NEG = -1e9
SHIFT = 10.0
SQRT_INV_T = float(np.sqrt(10.0))

_CACHE: dict = {}
_RUN_KW: dict = {}
_LAST_EXEC_NS = None
_LAST_RES = None


def _ew(k):
    """e-columns for tile-row k: 128 diag + 128*(32 or 31) wide."""
    return 4224 if k < 4 else 4096


def _mm_chunks(b0, b1, base):
    """split [b0,b1) e-cols into matmul chunks: 512-grid + zt wrap."""
    out = []
    off = b0
    while off < b1:
        n = min(512 - (off % 512), b1 - off)
        zl = (base + off) % B
        n = min(n, B - zl)
        out.append((off, zl, n))
        off += n
    return out


def _colp_chunks(k):
    """512-wide colsum chunks: (chunk_idx, [(flat_off, src, src_off, n)]).
    flat layout per k: [0, ew) unmasked colsums, [ew, ew+1408) masked."""
    ew = _ew(k)
    cw = ew + MASKW
    out = []
    for i in range((cw + 511) // 512):
        f0, f1 = 512 * i, min(512 * i + 512, cw)
        segs = []
        if f0 < ew:
            segs.append((f0, "e", f0, min(f1, ew) - f0))
        if f1 > ew:
            s = max(f0, ew)
            segs.append((s, "junk", s - ew, f1 - s))
        out.append((i, segs))
    return out


def _strip_ap(t):
    """AP over partitions {0, 32, 64, 96} x full free dim of tile t."""
    a = t[:]
    return bass.AP(tensor=a.tensor, offset=a.offset,
                   ap=[[a.ap[0][0] * 32, 4]] + [list(p) for p in a.ap[1:]])


def _build_program():
    nc = bacc.Bacc("TRN2", target_bir_lowering=False, debug=False,
                   num_devices=NCORES)

    d_zt = nc.dram_tensor("zt", [128, B], BF16, kind="ExternalInput").ap()
    d_tri = nc.dram_tensor("tri", [128, 128], F32, kind="ExternalInput").ap()
    d_mask = nc.dram_tensor("mask", [128, K, MASKW], BF16,
                            kind="ExternalInput").ap()
    # res[p, k] = row-direction s_all partial, res[p, 8+k] = s_pos partial
    d_res = nc.dram_tensor("res", [128, 2 * K], F32,
                           kind="ExternalOutput").ap()
    # colp[k, j, 512*b:+512]: chunk i = 4b + j of the per-k flat colsum
    # vector ([0, ew): unmasked, [ew, ew+1408): masked)
    d_colp = nc.dram_tensor("colp", [K, 4, 1536], F32,
                            kind="ExternalOutput").ap()

    with tile.TileContext(nc) as tc, ExitStack() as ctx:
        consts = ctx.enter_context(tc.tile_pool(name="consts", bufs=1))
        big = ctx.enter_context(tc.tile_pool(name="big", bufs=1))
        epool = ctx.enter_context(tc.tile_pool(name="epool", bufs=2))
        mpool = ctx.enter_context(tc.tile_pool(name="mpool", bufs=2))
        jpool = ctx.enter_context(tc.tile_pool(name="jpool", bufs=2))
        j2pool = ctx.enter_context(tc.tile_pool(name="j2pool", bufs=2))
        stats = ctx.enter_context(tc.tile_pool(name="stats", bufs=2))
        psp = ctx.enter_context(tc.tile_pool(name="psp", bufs=2, space="PSUM"))
        psc = ctx.enter_context(tc.tile_pool(name="psc", bufs=1, space="PSUM"))
        cstg = ctx.enter_context(tc.tile_pool(name="cstg", bufs=2))

        tri_neg = consts.tile([128, 128], F32)
        nc.sync.dma_start(out=tri_neg[:], in_=d_tri[:])
        bias_shift = consts.tile([128, 1], F32)
        nc.gpsimd.memset(bias_shift[:], -SHIFT)
        ones = consts.tile([128, 1], BF16)
        nc.gpsimd.memset(ones[:], 1.0)

        zt = big.tile([128, B], BF16)
        res = big.tile([128, 2 * K], F32)
        for h in range(8):
            nc.sync.dma_start(out=zt[:, h * 1024:(h + 1) * 1024],
                              in_=d_zt[:, h * 1024:(h + 1) * 1024])

        for k in range(K):
            ewk = _ew(k)
            base = 8 * k * 128
            lhsT = zt[:, base:base + 128]
            nblk = (ewk + 1023) // 1024
            e = epool.tile([128, 4224], BF16, tag="e")
            m = mpool.tile([128, MASKW], BF16, tag="m")
            nc.sync.dma_start(out=m[:], in_=d_mask[:, k, :])
            sacc = stats.tile([128, 8], F32, tag="sacc")

            for bi in range(nblk):
                b0 = bi * 1024
                b1 = min(b0 + 1024, ewk)
                bw = b1 - b0
                ps = psp.tile([128, 1024], F32, tag="ps")
                for off, zl, n in _mm_chunks(b0, b1, base):
                    nc.tensor.matmul(ps[:, off - b0:off - b0 + n],
                                     lhsT=lhsT, rhs=zt[:, zl:zl + n],
                                     start=True, stop=True)
                if bi == 0:
                    nc.vector.tensor_add(ps[:, 0:128], ps[:, 0:128],
                                         tri_neg[:])
                if bi < 3:
                    nc.scalar.activation(out=e[:, b0:b1], in_=ps[:, :bw],
                                         func=mybir.ActivationFunctionType.Exp,
                                         bias=bias_shift[:], scale=1.0,
                                         accum_out=sacc[:, bi:bi + 1])
                else:
                    nc.scalar.activation(out=e[:, b0:b1], in_=ps[:, :bw],
                                         func=mybir.ActivationFunctionType.Exp,
                                         bias=bias_shift[:], scale=1.0)
                    junk2 = j2pool.tile([128, 1024], BF16, tag="junk2")
                    nc.vector.tensor_scalar(
                        out=junk2[:, :bw], in0=e[:, b0:b1],
                        scalar1=1.0, scalar2=None, op0=mybir.AluOpType.mult,
                        op1=mybir.AluOpType.add,
                        accum_out=sacc[:, bi:bi + 1])

            # numerator row-direction: fused mask*e + row-sum on the band
            junk = jpool.tile([128, MASKW], BF16, tag="junk")
            nc.vector.scalar_tensor_tensor(
                out=junk[:], in0=m[:], scalar=1.0, in1=e[:, 0:MASKW],
                op0=mybir.AluOpType.mult, op1=mybir.AluOpType.mult,
                accum_out=res[:, K + k:K + k + 1])
            nc.vector.tensor_reduce(out=res[:, k:k + 1],
                                    in_=sacc[:, :nblk],
                                    axis=mybir.AxisListType.X,
                                    op=mybir.AluOpType.add)

            # column-direction sums: M=1 ones-matmuls packed 4 chunks per
            # PSUM bank via tile_position col-strips; one copy + DMA per k
            cb = psc.tile([128, 1536], F32, tag="cb")
            for i, segs in _colp_chunks(k):
                j = (i % 4) * 32
                sb = 512 * (i // 4)
                for f, src, so, n in segs:
                    rhs = (e[:, so:so + n] if src == "e"
                           else junk[:, so:so + n])
                    o = sb + (f - 512 * i)
                    nc.tensor.matmul(cb[j:j + 1, o:o + n],
                                     lhsT=ones[:], rhs=rhs,
                                     tile_position=(0, j),
                                     start=True, stop=True)
            cs = cstg.tile([128, 1536], F32, tag="cs")
            nc.vector.tensor_copy(cs[:], cb[:])
            nc.sync.dma_start(out=d_colp[k, :, :], in_=_strip_ap(cs))

        nc.sync.dma_start(out=d_res[:], in_=res[:])

    nc.compile()
    return nc


def _get_program():
    if "nc" not in _CACHE:
        _CACHE["nc"] = _build_program()
    return _CACHE["nc"]


def _host_prep(emb, t_i):
    perm = np.argsort(t_i, kind="stable")
    t_s = t_i[perm]
    emb_s = emb[perm]
    nrm = np.maximum(np.sqrt((emb_s.astype(np.float64) ** 2).sum(axis=1)),
                     1e-12)
    z = emb_s / nrm[:, None]
    zT = np.ascontiguousarray((z * SQRT_INV_T).T.astype(ml_dtypes.bfloat16))
    tri = np.where(np.arange(128)[:, None] >= np.arange(128)[None, :],
                   np.float32(NEG), np.float32(0.0))
    return perm, t_s, z, zT, tri


def _in_maps(zT, t_s, tri):
    t_sf = t_s.astype(np.float32)
    col_idx = np.arange(MASKW)
    maps = []
    for c in range(NCORES):
        zt_c = np.ascontiguousarray(np.roll(zT, -c * 128, axis=1))
        mask_c = np.empty((128, K, MASKW), dtype=ml_dtypes.bfloat16)
        for k in range(K):
            g0 = (8 * k + c) * 128
            rows = t_sf[g0:g0 + 128]
            cols = t_sf[(g0 + col_idx) % B]
            mask_c[:, k, :] = (
                np.abs(rows[:, None] - cols[None, :]) < 365.0
            ).astype(ml_dtypes.bfloat16)
        maps.append({"zt": zt_c, "mask": mask_c,
                     "tri": np.ascontiguousarray(tri)})
    return maps


def _combine(results, t_s, cen_s, z):
    s_all = np.zeros(B, np.float64)
    s_pos = np.zeros(B, np.float64)
    for c in range(NCORES):
        r = np.asarray(results[c]["res"], dtype=np.float64)
        craw = np.asarray(results[c]["colp"], dtype=np.float64)
        # [K, 4, 1536] -> flat [K, 6144]: chunk i=4b+j at [k, j, 512b:]
        strips = craw.reshape(K, 4, 3, 512)
        colp = strips.transpose(0, 2, 1, 3).reshape(K, -1)
        for k in range(K):
            g0 = (8 * k + c) * 128
            rows = g0 + np.arange(128)
            s_all[rows] += r[:, k]
            s_pos[rows] += r[:, K + k]
            ewk = _ew(k)
            cols = (g0 + np.arange(ewk)) % B
            s_all[cols] += colp[k, 0:ewk]
            mcols = (g0 + np.arange(MASKW)) % B
            s_pos[mcols] += colp[k, ewk:ewk + MASKW]

    lo = np.searchsorted(t_s, t_s - 364, side="left")
    hi = np.searchsorted(t_s, t_s + 364, side="right")
    has_pos = ((hi - lo - 1) > 0) & (cen_s == 1)

    g = np.arange(B)
    bad = has_pos & ((lo < g - MARGIN) | (hi > g + 1 + MARGIN))
    if np.any(bad):
        zs = (z * SQRT_INV_T).astype(np.float32)
        for i in np.nonzero(bad)[0]:
            extra = list(range(lo[i], i - MARGIN)) + \
                    list(range(i + 1 + MARGIN, hi[i]))
            js = np.array([j for j in extra if j != i], dtype=np.int64)
            if js.size:
                sims = zs[i] @ zs[js].T
                s_pos[i] += np.exp(sims - SHIFT).sum()

    cnt = float(has_pos.sum())
    if cnt <= 0:
        return np.float32(0.0)
    ratio = np.where(has_pos, s_all / np.maximum(s_pos, 1e-300), 1.0)
    per_row = np.where(has_pos, np.log(ratio), 0.0)
    return np.float32(per_row.sum() / max(cnt, 1.0))


def kernel(embeddings, survival_times, censor):
    emb = np.asarray(embeddings, dtype=np.float32)
    t_i = np.asarray(survival_times).astype(np.int64)
    cen = np.asarray(censor).astype(np.int64)
    assert emb.shape == (B, D)

    perm, t_s, z, zT, tri = _host_prep(emb, t_i)
    cen_s = cen[perm]
    nc = _get_program()
    maps = _in_maps(zT, t_s, tri)
    res = bass_utils.run_bass_kernel_spmd(nc, maps,
                                          core_ids=list(range(NCORES)),
                                          **_RUN_KW)
    global _LAST_EXEC_NS, _LAST_RES
    _LAST_EXEC_NS = res.exec_time_ns
    _LAST_RES = res
    return _combine(res.results, t_s, cen_s, z)
